# revision 21
# baseline (speedup 1.0000x reference)
"""LoRA layer kernel for Trainium2, SPMD across 8 NeuronCores.

Computes: out[b,s,h,d] = x[b,s,:] @ W_orig[:,h,d] + SCALE * (x @ A) @ B[:,h,d]

Strategy:
  - LoRA is folded on the host: W_eff = W + SCALE * (A @ B)  (exact
    algebraic identity — standard LoRA weight merge). The device kernel
    is then a pure GEMM out[8192, 2048] = x @ W_eff.
  - Data-parallel over tokens: 8192 tokens -> 1024 per core; W_eff
    replicated per core.
  - Mixed precision: 2 of 16 k-slabs (256 of 2048 contraction) run as
    one fp8e4m3 DoubleRow matmul per output chunk (2 rows/cycle), the
    rest in bf16. Exact-sim rel err 1.46e-2 < 2e-2 gate. The DR
    matmuls close each accumulation group (not open it): mid-stream
    their 256-col LDWEIGHTS hides under the preceding matmul; at a
    pass boundary a semaphore wait would block the pull-ahead.
  - Loop: chunk-pair (1024 output cols) outer, token tiles inner
    (triple first pass, then pairs): >=4 N=512 matmuls share each
    256 KiB W slab so compute outpaces the W stream.
  - All input DMAs on one HWDGE ring in consumption order (a second
    ring competes for HBM and starves the W stream); early slabs ship
    as single-slab DMAs because the completion SEMAPHORE lags the data
    by ~1.5-2 us — small first transfers gate compute sooner.
  - PE prewarm: dummy matmuls during the DMA-fill window lift the HAM
    clock gate to 8/8 before real work arrives.
  - PSUM accumulates fp32, output staged bf16, upcast to fp32 on host.
"""

import numpy as np

# Problem shapes (hardcoded per contract - kernel.py must be self-contained)
B, S, H = 4, 2048, 2048
NH, HD = 16, 128
N = NH * HD            # 2048 output features
RANK = 4
ALPHA = 4.0
SCALE = ALPHA / RANK   # 1.0
NCORES = 8
TOK = B * S            # 8192 tokens total
TPC = TOK // NCORES    # 1024 tokens per core

P = 128                # SBUF partitions
KT = H // P            # 16 contraction slabs
KF8 = 2                # trailing slabs done in fp8 DoubleRow
KTB = KT - KF8         # bf16 slabs
TT = TPC // P          # 8 token tiles per core
CH = 512               # psum bank width (fp32)
NCP = 2                # chunk-pairs (1024 cols each)

_CACHE = {}


def _build_program():
    import concourse.mybir as mybir
    import concourse.tile as tile
    from concourse import bacc

    f32 = mybir.dt.float32
    bf16 = mybir.dt.bfloat16
    f8 = mybir.dt.float8e4
    DR = mybir.MatmulPerfMode.DoubleRow

    nc = bacc.Bacc(None, target_bir_lowering=False, debug=False)

    x8d = nc.dram_tensor("x8", [P, TT, KF8, P], f8, kind="ExternalInput")
    w8d = nc.dram_tensor("w8", [NCP, P, KF8, 2 * CH], f8, kind="ExternalInput")
    xt = nc.dram_tensor("xt", [P, TT, KTB, P], bf16, kind="ExternalInput")
    w = nc.dram_tensor("w", [NCP, P, KTB, 2 * CH], bf16, kind="ExternalInput")
    out = nc.dram_tensor("out", [TPC, N], bf16, kind="ExternalOutput")

    with tile.TileContext(nc) as tc:
        with (
            tc.tile_pool(name="wpool", bufs=1) as wpool,
            tc.tile_pool(name="xpool", bufs=1) as xpool,
            tc.tile_pool(name="opool", bufs=4) as opool,
            tc.tile_pool(name="psum", bufs=1, space="PSUM") as psum,
        ):
            w_sb = [
                wpool.tile([P, KTB, 2 * CH], bf16, tag=f"w{cp}", name=f"w_{cp}")
                for cp in range(NCP)
            ]
            w8_sb = [
                wpool.tile([P, KF8, 2 * CH], f8, tag=f"w8{cp}", name=f"w8_{cp}")
                for cp in range(NCP)
            ]
            x_sb = xpool.tile([P, TT, KTB, P], bf16, tag="x", name="x_sb")
            x8_sb = xpool.tile([P, TT, KF8, P], f8, tag="x8", name="x8_sb")
            warm = xpool.tile([P, CH], bf16, tag="warm", name="warm")

            # ---- input DMAs: one ring, consumption order; early W slabs as
            # singles so their completion sems land before compute needs them
            nc.sync.dma_start(x_sb[:, 0:1], xt[:, 0:1])
            nc.sync.dma_start(w_sb[0][:, 0:1], w[0, :, 0:1])
            nc.sync.dma_start(x_sb[:, 1:2], xt[:, 1:2])
            nc.sync.dma_start(w_sb[0][:, 1:2], w[0, :, 1:2])
            nc.sync.dma_start(w_sb[0][:, 2:3], w[0, :, 2:3])
            nc.sync.dma_start(x_sb[:, 2:3], xt[:, 2:3])
            nc.sync.dma_start(w_sb[0][:, 3:4], w[0, :, 3:4])
            nc.sync.dma_start(w_sb[0][:, 4:5], w[0, :, 4:5])
            nc.sync.dma_start(x_sb[:, 3:4], xt[:, 3:4])
            nc.sync.dma_start(w_sb[0][:, 5:7], w[0, :, 5:7])
            nc.sync.dma_start(x8_sb[:], x8d[:])
            nc.sync.dma_start(w8_sb[0][:], w8d[0])
            nc.sync.dma_start(x_sb[:, 4:6], xt[:, 4:6])
            nc.sync.dma_start(w_sb[0][:, 7:9], w[0, :, 7:9])
            nc.sync.dma_start(w_sb[0][:, 9:11], w[0, :, 9:11])
            nc.sync.dma_start(x_sb[:, 6:8], xt[:, 6:8])
            nc.sync.dma_start(w_sb[0][:, 11:KTB], w[0, :, 11:KTB])
            nc.sync.dma_start(w8_sb[1][:], w8d[1])
            nc.sync.dma_start(w_sb[1][:, 0:7], w[1, :, 0:7])
            nc.sync.dma_start(w_sb[1][:, 7:KTB], w[1, :, 7:KTB])

            # ---- PE prewarm: dummy matmuls while input DMAs are in flight,
            # so the HAM clock gate reaches 8/8 before the first real MM and
            # the cold 1.2 GHz window burns idle time instead of real work.
            nc.vector.memset(warm[:], 0.0)
            # prime the scalar engine's activation table while it's idle, so
            # the final close's scalar copy doesn't pay the ~1.3us table load
            nc.scalar.copy(warm[:, CH - 4:CH], warm[:, 0:4])
            qw = psum.tile([P, CH], f32, tag="q0", name="q_warm")
            for _ in range(40):
                nc.tensor.matmul(qw[:, 0:P], warm[:, 0:P], warm[:, 0:P],
                                 start=True, stop=True)
            # a few wide dummies stretch the warm window over the first W
            # slabs' DMA-semaphore latency, absorbing the startup stall
            for _ in range(10):
                nc.tensor.matmul(qw[:], warm[:, 0:P], warm[:],
                                 start=True, stop=True)

            # ---- compute: chunk-pair outer, token tiles inner ----
            def close(cp, t, q):
                ot = opool.tile([P, 2 * CH], bf16, tag="o", name=f"o_{cp}_{t}")
                nc.vector.tensor_copy(ot[:], q[:])
                nc.scalar.dma_start(
                    out[t * P:(t + 1) * P, cp * 2 * CH:(cp + 1) * 2 * CH],
                    ot[:])

            def mm_dr(q, t, wt8):
                """fp8 DoubleRow: both trailing k-slabs in one matmul/chunk;
                closes the accumulation group (stop=True)."""
                a = x8_sb[:, t, 0:KF8, :]
                nc.tensor.matmul(q[:, 0:CH], a, wt8[:, 0:KF8, 0:CH],
                                 start=False, stop=True, perf_mode=DR)
                nc.tensor.matmul(q[:, CH:2 * CH], a, wt8[:, 0:KF8, CH:2 * CH],
                                 start=False, stop=True, perf_mode=DR)

            def mm(q, t, k, wt, st):
                lhsT = x_sb[:, t, k, :]
                nc.tensor.matmul(q[:, 0:CH], lhsT, wt[:, k, 0:CH],
                                 start=st, stop=False)
                nc.tensor.matmul(q[:, CH:2 * CH], lhsT, wt[:, k, CH:2 * CH],
                                 start=st, stop=False)

            def open_q(cp, t, tag):
                return psum.tile([P, 2 * CH], f32, tag=tag, name=f"q_{cp}_{t}")

            def finish(cp, t, q):
                mm_dr(q, t, w8_sb[cp])
                close(cp, t, q)

            def run_pair(cp, t0, t1, g0, g1):
                """One pass: tiles (t0, t1) over all k for chunk-pair cp."""
                q0 = open_q(cp, t0, g0)
                q1 = open_q(cp, t1, g1)
                wt = w_sb[cp]
                for k in range(KTB):
                    mm(q0, t0, k, wt, k == 0)
                    mm(q1, t1, k, wt, k == 0)
                finish(cp, t0, q0)
                finish(cp, t1, q1)

            def run_triple(cp, ts, gs, lag=2):
                """Startup pass: 3 tiles staggered by `lag` k-slabs — 6 MMs
                per W slab keeps consumption below the DMA delivery rate, so
                the PE never outruns the W stream; staggering spreads the
                closes so the next pass's psum frees early."""
                t0, t1, t2 = ts
                q = [open_q(cp, t, g) for t, g in zip(ts, gs)]
                wt = w_sb[cp]
                for k in range(lag):
                    mm(q[0], t0, k, wt, k == 0)
                for k in range(lag, 2 * lag):
                    mm(q[0], t0, k, wt, False)
                    mm(q[1], t1, k - lag, wt, k - lag == 0)
                for k in range(2 * lag, KTB):
                    mm(q[0], t0, k, wt, False)
                    mm(q[1], t1, k - lag, wt, False)
                    mm(q[2], t2, k - 2 * lag, wt, k - 2 * lag == 0)
                finish(cp, t0, q[0])
                for k in range(KTB - lag, KTB):
                    mm(q[1], t1, k, wt, False)
                    mm(q[2], t2, k - lag, wt, False)
                finish(cp, t1, q[1])
                for k in range(KTB - lag, KTB):
                    mm(q[2], t2, k, wt, False)
                finish(cp, t2, q[2])

            def run_single(cp, t, g):
                q = open_q(cp, t, g)
                wt = w_sb[cp]
                for k in range(KTB):
                    mm(q, t, k, wt, k == 0)
                finish(cp, t, q)

            def run_pair_lagged(cp, t0, t1, g0, g1, lag=3):
                """Last pass: t1 lags so t0's close overlaps t1's tail MMs.
                t1's two psum banks are SEPARATE tiles so bank A's copy can
                run concurrently with bank B's final matmul (a shared tile
                serializes them via a coarse WAR dependency)."""
                q0 = open_q(cp, t0, g0)
                qa = psum.tile([P, CH], f32, tag="q0", name=f"qa_{cp}_{t1}")
                qb = psum.tile([P, CH], f32, tag="q1", name=f"qb_{cp}_{t1}")
                wt = w_sb[cp]

                def mm1(k, st):
                    lhsT = x_sb[:, t1, k, :]
                    nc.tensor.matmul(qa[:], lhsT, wt[:, k, 0:CH],
                                     start=st, stop=False)
                    nc.tensor.matmul(qb[:], lhsT, wt[:, k, CH:2 * CH],
                                     start=st, stop=False)

                for k in range(lag):
                    mm(q0, t0, k, wt, k == 0)
                for k in range(lag, KTB):
                    mm(q0, t0, k, wt, False)
                    mm1(k - lag, k - lag == 0)
                finish(cp, t0, q0)
                for k in range(KTB - lag, KTB):
                    mm1(k, False)
                # per-bank close: copy+DMA of bank A overlap bank B's DR MM
                a = x8_sb[:, t1, 0:KF8, :]
                wt8 = w8_sb[cp]
                ot = opool.tile([P, 2 * CH], bf16, tag="o", name=f"o_{cp}_{t1}")
                nc.tensor.matmul(qa[:], a, wt8[:, 0:KF8, 0:CH],
                                 start=False, stop=True, perf_mode=DR)
                nc.tensor.matmul(qb[:], a, wt8[:, 0:KF8, CH:2 * CH],
                                 start=False, stop=True, perf_mode=DR)
                nc.vector.tensor_copy(ot[:, 0:CH], qa[:])
                nc.scalar.copy(ot[:, CH:2 * CH], qb[:])
                nc.scalar.dma_start(
                    out[t1 * P:(t1 + 1) * P, cp * 2 * CH:cp * 2 * CH + CH],
                    ot[:, 0:CH])
                nc.scalar.dma_start(
                    out[t1 * P:(t1 + 1) * P,
                        cp * 2 * CH + CH:(cp + 1) * 2 * CH],
                    ot[:, CH:2 * CH])

            run_triple(0, (0, 1, 2), ("q0", "q1", "q2"))
            run_pair(0, 3, 4, "q3", "q0")
            run_pair(0, 5, 6, "q1", "q2")
            run_single(0, 7, "q3")
            run_pair(1, 0, 1, "q0", "q1")
            run_pair(1, 2, 3, "q2", "q3")
            run_pair(1, 4, 5, "q0", "q1")
            run_pair_lagged(1, 6, 7, "q2", "q3")

    nc.compile()
    return nc


def _prep_inputs(x, W_orig, A_kernel, B_kernel):
    import ml_dtypes

    bf16 = ml_dtypes.bfloat16
    f8 = ml_dtypes.float8_e4m3
    x = np.asarray(x, dtype=np.float32)
    W_orig = np.asarray(W_orig, dtype=np.float32)
    A_kernel = np.asarray(A_kernel, dtype=np.float32)
    B_kernel = np.asarray(B_kernel, dtype=np.float32)

    # Fold the LoRA update into the dense weight (exact identity):
    #   x@W + SCALE*(x@A)@B  ==  x @ (W + SCALE*A@B)
    W2 = W_orig.reshape(H, N)
    W_eff = W2 + np.float32(SCALE) * (A_kernel @ B_kernel.reshape(RANK, N))
    KB = KF8 * P
    # fp8 slabs are the LAST KF8 k-slabs; bf16 slabs the first KTB
    w8 = np.ascontiguousarray(
        W_eff[H - KB:].reshape(KF8, P, NCP, 2 * CH).transpose(2, 1, 0, 3)
        .astype(f8))
    w4 = np.ascontiguousarray(
        W_eff[:H - KB].reshape(KTB, P, NCP, 2 * CH).transpose(2, 1, 0, 3)
        .astype(bf16))

    x2d = x.reshape(TOK, H)
    in_maps = []
    for i in range(NCORES):
        xs = x2d[i * TPC:(i + 1) * TPC]                    # [TPC, H]
        # x8[p, t, ks, j] = xs[t*128 + j, (KTB + ks)*128 + p]
        x8c = np.ascontiguousarray(
            xs[:, H - KB:].reshape(TT, P, KF8, P).transpose(3, 0, 2, 1)
            .astype(f8))
        # xt[p, t, k, j] = xs[t*128 + j, k*128 + p], k < KTB
        xtc = np.ascontiguousarray(
            xs[:, :H - KB].reshape(TT, P, KTB, P).transpose(3, 0, 2, 1)
            .astype(bf16))
        in_maps.append({"x8": x8c, "xt": xtc, "w8": w8, "w": w4})
    return in_maps


def kernel(x, W_orig, A_kernel, B_kernel):
    from concourse.bass_utils import run_bass_kernel_spmd

    if "nc" not in _CACHE:
        _CACHE["nc"] = _build_program()
    nc = _CACHE["nc"]

    in_maps = _prep_inputs(x, W_orig, A_kernel, B_kernel)
    res = run_bass_kernel_spmd(nc, in_maps, list(range(NCORES)))
    parts = [np.asarray(res.results[i]["out"]) for i in range(NCORES)]
    full = np.concatenate(parts, axis=0).astype(np.float32)   # [TOK, N]
    return full.reshape(B, S, NH, HD)


# revision 24
# speedup vs baseline: 1.0085x; 1.0085x over previous
"""LoRA layer kernel for Trainium2, SPMD across 8 NeuronCores.

Computes: out[b,s,h,d] = x[b,s,:] @ W_orig[:,h,d] + SCALE * (x @ A) @ B[:,h,d]

Strategy:
  - LoRA is folded on the host: W_eff = W + SCALE * (A @ B)  (exact
    algebraic identity — standard LoRA weight merge). The device kernel
    is then a pure GEMM out[8192, 2048] = x @ W_eff.
  - Data-parallel over tokens: 8192 tokens -> 1024 per core; W_eff
    replicated per core.
  - Mixed precision: 2 of 16 k-slabs (256 of 2048 contraction) run as
    one fp8e4m3 DoubleRow matmul per output chunk (2 rows/cycle), the
    rest in bf16. Exact-sim rel err 1.46e-2 < 2e-2 gate. The DR
    matmuls close each accumulation group (not open it): mid-stream
    their 256-col LDWEIGHTS hides under the preceding matmul; at a
    pass boundary a semaphore wait would block the pull-ahead.
  - Loop: chunk-pair (1024 output cols) outer, token tiles inner
    (triple first pass, then pairs): >=4 N=512 matmuls share each
    256 KiB W slab so compute outpaces the W stream.
  - All input DMAs on one HWDGE ring in consumption order (a second
    ring competes for HBM and starves the W stream); early slabs ship
    as single-slab DMAs because the completion SEMAPHORE lags the data
    by ~1.5-2 us — small first transfers gate compute sooner.
  - PE prewarm: dummy matmuls during the DMA-fill window lift the HAM
    clock gate to 8/8 before real work arrives.
  - PSUM accumulates fp32, output staged bf16, upcast to fp32 on host.
"""

import numpy as np

# Problem shapes (hardcoded per contract - kernel.py must be self-contained)
B, S, H = 4, 2048, 2048
NH, HD = 16, 128
N = NH * HD            # 2048 output features
RANK = 4
ALPHA = 4.0
SCALE = ALPHA / RANK   # 1.0
NCORES = 8
TOK = B * S            # 8192 tokens total
TPC = TOK // NCORES    # 1024 tokens per core

P = 128                # SBUF partitions
KT = H // P            # 16 contraction slabs
KF8 = 2                # trailing slabs done in fp8 DoubleRow
KTB = KT - KF8         # bf16 slabs
TT = TPC // P          # 8 token tiles per core
CH = 512               # psum bank width (fp32)
NCP = 2                # chunk-pairs (1024 cols each)

_CACHE = {}


def _build_program():
    import concourse.mybir as mybir
    import concourse.tile as tile
    from concourse import bacc

    f32 = mybir.dt.float32
    bf16 = mybir.dt.bfloat16
    f8 = mybir.dt.float8e4
    DR = mybir.MatmulPerfMode.DoubleRow

    nc = bacc.Bacc(None, target_bir_lowering=False, debug=False)

    x8d = nc.dram_tensor("x8", [P, TT, KF8, P], f8, kind="ExternalInput")
    w8d = nc.dram_tensor("w8", [NCP, P, KF8, 2 * CH], f8, kind="ExternalInput")
    xt = nc.dram_tensor("xt", [P, TT, KTB, P], bf16, kind="ExternalInput")
    w = nc.dram_tensor("w", [NCP, P, KTB, 2 * CH], bf16, kind="ExternalInput")
    out = nc.dram_tensor("out", [TPC, N], bf16, kind="ExternalOutput")

    with tile.TileContext(nc) as tc:
        with (
            tc.tile_pool(name="wpool", bufs=1) as wpool,
            tc.tile_pool(name="xpool", bufs=1) as xpool,
            tc.tile_pool(name="opool", bufs=4) as opool,
            tc.tile_pool(name="psum", bufs=1, space="PSUM") as psum,
        ):
            w_sb = [
                wpool.tile([P, KTB, 2 * CH], bf16, tag=f"w{cp}", name=f"w_{cp}")
                for cp in range(NCP)
            ]
            w8_sb = [
                wpool.tile([P, KF8, 2 * CH], f8, tag=f"w8{cp}", name=f"w8_{cp}")
                for cp in range(NCP)
            ]
            x_sb = xpool.tile([P, TT, KTB, P], bf16, tag="x", name="x_sb")
            x8_sb = xpool.tile([P, TT, KF8, P], f8, tag="x8", name="x8_sb")
            warm = xpool.tile([P, CH], bf16, tag="warm", name="warm")

            # ---- input DMAs: one ring, consumption order; early W slabs as
            # singles so their completion sems land before compute needs them
            nc.sync.dma_start(x_sb[:, 0:1], xt[:, 0:1])
            nc.sync.dma_start(w_sb[0][:, 0:1], w[0, :, 0:1])
            nc.sync.dma_start(x_sb[:, 1:2], xt[:, 1:2])
            nc.sync.dma_start(w_sb[0][:, 1:2], w[0, :, 1:2])
            nc.sync.dma_start(w_sb[0][:, 2:3], w[0, :, 2:3])
            nc.sync.dma_start(x_sb[:, 2:3], xt[:, 2:3])
            nc.sync.dma_start(w_sb[0][:, 3:4], w[0, :, 3:4])
            nc.sync.dma_start(w_sb[0][:, 4:5], w[0, :, 4:5])
            nc.sync.dma_start(w_sb[0][:, 5:7], w[0, :, 5:7])
            nc.sync.dma_start(x_sb[:, 3:4], xt[:, 3:4])
            nc.sync.dma_start(x8_sb[:], x8d[:])
            nc.sync.dma_start(w8_sb[0][:], w8d[0])
            nc.sync.dma_start(x_sb[:, 4:6], xt[:, 4:6])
            nc.sync.dma_start(w_sb[0][:, 7:9], w[0, :, 7:9])
            nc.sync.dma_start(w_sb[0][:, 9:11], w[0, :, 9:11])
            nc.sync.dma_start(x_sb[:, 6:8], xt[:, 6:8])
            nc.sync.dma_start(w_sb[0][:, 11:KTB], w[0, :, 11:KTB])
            nc.sync.dma_start(w8_sb[1][:], w8d[1])
            nc.sync.dma_start(w_sb[1][:, 0:7], w[1, :, 0:7])
            nc.sync.dma_start(w_sb[1][:, 7:KTB], w[1, :, 7:KTB])

            # ---- PE prewarm: dummy matmuls while input DMAs are in flight,
            # so the HAM clock gate reaches 8/8 before the first real MM and
            # the cold 1.2 GHz window burns idle time instead of real work.
            nc.vector.memset(warm[:], 0.0)
            # prime the scalar engine's activation table while it's idle, so
            # the final close's scalar copy doesn't pay the ~1.3us table load
            nc.scalar.copy(warm[:, CH - 4:CH], warm[:, 0:4])
            qw = psum.tile([P, CH], f32, tag="q0", name="q_warm")
            for _ in range(40):
                nc.tensor.matmul(qw[:, 0:P], warm[:, 0:P], warm[:, 0:P],
                                 start=True, stop=True)
            # a few wide dummies stretch the warm window over the first W
            # slabs' DMA-semaphore latency, absorbing the startup stall
            for _ in range(5):
                nc.tensor.matmul(qw[:], warm[:, 0:P], warm[:],
                                 start=True, stop=True)

            # ---- compute: chunk-pair outer, token tiles inner ----
            def close(cp, t, q):
                ot = opool.tile([P, 2 * CH], bf16, tag="o", name=f"o_{cp}_{t}")
                nc.vector.tensor_copy(ot[:], q[:])
                nc.scalar.dma_start(
                    out[t * P:(t + 1) * P, cp * 2 * CH:(cp + 1) * 2 * CH],
                    ot[:])

            def mm_dr(q, t, wt8):
                """fp8 DoubleRow: both trailing k-slabs in one matmul/chunk;
                closes the accumulation group (stop=True)."""
                a = x8_sb[:, t, 0:KF8, :]
                nc.tensor.matmul(q[:, 0:CH], a, wt8[:, 0:KF8, 0:CH],
                                 start=False, stop=True, perf_mode=DR)
                nc.tensor.matmul(q[:, CH:2 * CH], a, wt8[:, 0:KF8, CH:2 * CH],
                                 start=False, stop=True, perf_mode=DR)

            def mm(q, t, k, wt, st):
                lhsT = x_sb[:, t, k, :]
                nc.tensor.matmul(q[:, 0:CH], lhsT, wt[:, k, 0:CH],
                                 start=st, stop=False)
                nc.tensor.matmul(q[:, CH:2 * CH], lhsT, wt[:, k, CH:2 * CH],
                                 start=st, stop=False)

            def open_q(cp, t, tag):
                return psum.tile([P, 2 * CH], f32, tag=tag, name=f"q_{cp}_{t}")

            def finish(cp, t, q):
                mm_dr(q, t, w8_sb[cp])
                close(cp, t, q)

            def run_pair(cp, t0, t1, g0, g1):
                """One pass: tiles (t0, t1) over all k for chunk-pair cp."""
                q0 = open_q(cp, t0, g0)
                q1 = open_q(cp, t1, g1)
                wt = w_sb[cp]
                for k in range(KTB):
                    mm(q0, t0, k, wt, k == 0)
                    mm(q1, t1, k, wt, k == 0)
                finish(cp, t0, q0)
                finish(cp, t1, q1)

            def run_triple(cp, ts, gs, lag=2):
                """Startup pass: 3 tiles staggered by `lag` k-slabs — 6 MMs
                per W slab keeps consumption below the DMA delivery rate, so
                the PE never outruns the W stream; staggering spreads the
                closes so the next pass's psum frees early."""
                t0, t1, t2 = ts
                q = [open_q(cp, t, g) for t, g in zip(ts, gs)]
                wt = w_sb[cp]
                for k in range(lag):
                    mm(q[0], t0, k, wt, k == 0)
                for k in range(lag, 2 * lag):
                    mm(q[0], t0, k, wt, False)
                    mm(q[1], t1, k - lag, wt, k - lag == 0)
                for k in range(2 * lag, KTB):
                    mm(q[0], t0, k, wt, False)
                    mm(q[1], t1, k - lag, wt, False)
                    mm(q[2], t2, k - 2 * lag, wt, k - 2 * lag == 0)
                finish(cp, t0, q[0])
                for k in range(KTB - lag, KTB):
                    mm(q[1], t1, k, wt, False)
                    mm(q[2], t2, k - lag, wt, False)
                finish(cp, t1, q[1])
                for k in range(KTB - lag, KTB):
                    mm(q[2], t2, k, wt, False)
                finish(cp, t2, q[2])

            def run_single(cp, t, g):
                q = open_q(cp, t, g)
                wt = w_sb[cp]
                for k in range(KTB):
                    mm(q, t, k, wt, k == 0)
                finish(cp, t, q)

            def run_pair_lagged(cp, t0, t1, g0, g1, lag=3):
                """Last pass: t1 lags so t0's close overlaps t1's tail MMs.
                t1's two psum banks are SEPARATE tiles so bank A's copy can
                run concurrently with bank B's final matmul (a shared tile
                serializes them via a coarse WAR dependency)."""
                q0 = open_q(cp, t0, g0)
                qa = psum.tile([P, CH], f32, tag="q0", name=f"qa_{cp}_{t1}")
                qb = psum.tile([P, CH], f32, tag="q1", name=f"qb_{cp}_{t1}")
                wt = w_sb[cp]

                def mm1(k, st):
                    lhsT = x_sb[:, t1, k, :]
                    nc.tensor.matmul(qa[:], lhsT, wt[:, k, 0:CH],
                                     start=st, stop=False)
                    nc.tensor.matmul(qb[:], lhsT, wt[:, k, CH:2 * CH],
                                     start=st, stop=False)

                for k in range(lag):
                    mm(q0, t0, k, wt, k == 0)
                for k in range(lag, KTB):
                    mm(q0, t0, k, wt, False)
                    mm1(k - lag, k - lag == 0)
                finish(cp, t0, q0)
                for k in range(KTB - lag, KTB):
                    mm1(k, False)
                # per-bank close: copy+DMA of bank A overlap bank B's DR MM
                a = x8_sb[:, t1, 0:KF8, :]
                wt8 = w8_sb[cp]
                ot = opool.tile([P, 2 * CH], bf16, tag="o", name=f"o_{cp}_{t1}")
                nc.tensor.matmul(qa[:], a, wt8[:, 0:KF8, 0:CH],
                                 start=False, stop=True, perf_mode=DR)
                nc.tensor.matmul(qb[:], a, wt8[:, 0:KF8, CH:2 * CH],
                                 start=False, stop=True, perf_mode=DR)
                nc.vector.tensor_copy(ot[:, 0:CH], qa[:])
                nc.scalar.copy(ot[:, CH:2 * CH], qb[:])
                nc.scalar.dma_start(
                    out[t1 * P:(t1 + 1) * P, cp * 2 * CH:cp * 2 * CH + CH],
                    ot[:, 0:CH])
                # split the final half across BOTH HWDGE rings so the last
                # bytes (and their completion sems) land sooner
                nc.sync.dma_start(
                    out[t1 * P:(t1 + 1) * P,
                        cp * 2 * CH + CH:cp * 2 * CH + CH + CH // 2],
                    ot[:, CH:CH + CH // 2])
                nc.scalar.dma_start(
                    out[t1 * P:(t1 + 1) * P,
                        cp * 2 * CH + CH + CH // 2:(cp + 1) * 2 * CH],
                    ot[:, CH + CH // 2:2 * CH])

            run_triple(0, (0, 1, 2), ("q0", "q1", "q2"))
            run_pair(0, 3, 4, "q3", "q0")
            run_pair(0, 5, 6, "q1", "q2")
            run_single(0, 7, "q3")
            run_pair(1, 0, 1, "q0", "q1")
            run_pair(1, 2, 3, "q2", "q3")
            run_pair(1, 4, 5, "q0", "q1")
            run_pair_lagged(1, 6, 7, "q2", "q3")

    nc.compile()
    return nc


def _prep_inputs(x, W_orig, A_kernel, B_kernel):
    import ml_dtypes

    bf16 = ml_dtypes.bfloat16
    f8 = ml_dtypes.float8_e4m3
    x = np.asarray(x, dtype=np.float32)
    W_orig = np.asarray(W_orig, dtype=np.float32)
    A_kernel = np.asarray(A_kernel, dtype=np.float32)
    B_kernel = np.asarray(B_kernel, dtype=np.float32)

    # Fold the LoRA update into the dense weight (exact identity):
    #   x@W + SCALE*(x@A)@B  ==  x @ (W + SCALE*A@B)
    W2 = W_orig.reshape(H, N)
    W_eff = W2 + np.float32(SCALE) * (A_kernel @ B_kernel.reshape(RANK, N))
    KB = KF8 * P
    # fp8 slabs are the LAST KF8 k-slabs; bf16 slabs the first KTB
    w8 = np.ascontiguousarray(
        W_eff[H - KB:].reshape(KF8, P, NCP, 2 * CH).transpose(2, 1, 0, 3)
        .astype(f8))
    w4 = np.ascontiguousarray(
        W_eff[:H - KB].reshape(KTB, P, NCP, 2 * CH).transpose(2, 1, 0, 3)
        .astype(bf16))

    x2d = x.reshape(TOK, H)
    in_maps = []
    for i in range(NCORES):
        xs = x2d[i * TPC:(i + 1) * TPC]                    # [TPC, H]
        # x8[p, t, ks, j] = xs[t*128 + j, (KTB + ks)*128 + p]
        x8c = np.ascontiguousarray(
            xs[:, H - KB:].reshape(TT, P, KF8, P).transpose(3, 0, 2, 1)
            .astype(f8))
        # xt[p, t, k, j] = xs[t*128 + j, k*128 + p], k < KTB
        xtc = np.ascontiguousarray(
            xs[:, :H - KB].reshape(TT, P, KTB, P).transpose(3, 0, 2, 1)
            .astype(bf16))
        in_maps.append({"x8": x8c, "xt": xtc, "w8": w8, "w": w4})
    return in_maps


def kernel(x, W_orig, A_kernel, B_kernel):
    from concourse.bass_utils import run_bass_kernel_spmd

    if "nc" not in _CACHE:
        _CACHE["nc"] = _build_program()
    nc = _CACHE["nc"]

    in_maps = _prep_inputs(x, W_orig, A_kernel, B_kernel)
    res = run_bass_kernel_spmd(nc, in_maps, list(range(NCORES)))
    parts = [np.asarray(res.results[i]["out"]) for i in range(NCORES)]
    full = np.concatenate(parts, axis=0).astype(np.float32)   # [TOK, N]
    return full.reshape(B, S, NH, HD)
